# revision 45
# baseline (speedup 1.0000x reference)
"""Dilated KNN (k=9, dilation=2) over query[4, 8192, 64] on 8 NeuronCores.

Sharding: batch b and query-half h per core (core = 2*b + h). Each core
computes scores s[m, n] = 2*x_m.x_n - |x_n|^2 for its 4096 queries against
all 8192 supports of its batch (same ranking as negated squared euclidean
distance), selects the top-17 per row, and emits indices of ranks
0, 2, ..., 16.

Host prep: the query-side hi/lo mantissa split (11/13 bits, exact
2ah+2al==2a) and the support norms (fp32 sum + hi/lo split) are computed in
numpy; the device setup is chunked DMA + one-pass ACT f32r casts, with the
support lo part (bl = b - bh) derived on-chip, so the main loop starts
within ~20us. (The PE's f32r rounding is coarse (~tf32): collapsing the
hi/lo split to a single K=64 pass was measured to fail the accuracy gate.)

Single-DVE-pass top-k ("iota-stamp"):
  PE   : fp32r hi/lo split matmuls (exact products, fp32 PSUM accumulate)
         MM1: [2ah; 2al] . [bh; bh]          (K=128)
         MM2: [2ah; 1; 1] . [bl; -sqh; -sql] (K=66, drops 2*al.bl ~ 1e-6)
  ACT  : evicts PSUM through a monotone Exp map y = exp(s - 42.8), so the
         fp32 value order equals the score order with uniform absolute
         resolution ~2^-23 in score units.
  Pool : gpsimd iota overwrites byte 0 of every fp32 y with (255 - li),
         li = column index within a 256-wide chunk. Ranking resolution
         drops to ~3e-5 score units (fine: adjacent top-17 gaps are ~1e-1),
         and every candidate now carries its position in its low bits.
  DVE  : one max8 per 256-chunk (32/tile) -> 256 candidates with embedded
         positions; 3 merge rounds (max8 + match_replace into fresh buffers)
         give the top-24; two max_index calls over the original candidates
         recover the chunks for the 9 ranks the dilated output needs
         (0,2,...,16): call 1 a stride-2 view of ranks 0..14, call 2 ranks
         16..23.
  Decode (batched, quarter of the tiles at a time):
         global = ((slot >> 3) << 8) + 255 - (bits & 0xFF).
"""

import sys
import types

import numpy as np

B = 4
N = 8192
C = 64
K_OUT = 9
NQ = N // 2
N_CORES = 8
CHUNK = 256          # max8 scan chunk == iota stamp period
N_CHUNKS = N // CHUNK
NEG_BIG = -1.0e38
EXP_SHIFT = 42.8     # y = exp(s - 42.8); relevant scores s in [-25, 111]
BLK = 2048           # PSUM eviction block (columns per ACT op)
N_BLK = N // BLK
LOAD_CHUNK = 512


def _install_ntff_shim():
    """bass_utils imports antenv.axon_hooks for trace=True; the agent image
    lacks it. Register the ctypes-based hook so NTFF profiling works."""
    if "antenv.axon_hooks" in sys.modules:
        return
    try:
        from trn_agent_boot.trn_boot import _ntff_profile_via_ctypes

        hook = _ntff_profile_via_ctypes("/opt/axon/libaxon_pjrt.so")
        m = types.ModuleType("antenv.axon_hooks")
        m.get_axon_ntff_profile_hook = lambda: hook
        sys.modules["antenv.axon_hooks"] = m
    except Exception:
        pass


def build_kernel(nc, n_queries=NQ):
    import concourse.mybir as mybir
    import concourse.tile as tile

    F32 = mybir.dt.float32
    F32R = mybir.dt.float32r
    U32 = mybir.dt.uint32
    U8 = mybir.dt.uint8
    I32 = mybir.dt.int32

    m_tiles = n_queries // 128
    # bns rows 0..63: raw support features b; row 64: -sqh; row 65: -sql
    bnsd = nc.dram_tensor("bns", [66, N], F32, kind="ExternalInput")
    # l1 rows 0..63: 2*ah; rows 64..127: 2*al (host 11-bit hi/lo split)
    l1d = nc.dram_tensor("l1", [128, n_queries], F32, kind="ExternalInput")
    out = nc.dram_tensor("idx", [n_queries, K_OUT], I32, kind="ExternalOutput")

    with tile.TileContext(nc) as tc:
        with (
            tc.tile_pool(name="const", bufs=1) as constp,
            tc.tile_pool(name="big", bufs=1) as bigp,
        ):
            bias_t = constp.tile([128, 1], F32)
            nc.vector.memset(bias_t[:, :], -EXP_SHIFT)
            ones2 = constp.tile([2, 512], F32)
            nc.vector.memset(ones2[:, :], 1.0)
            c3 = constp.tile([128, 1], U32)
            nc.vector.memset(c3[:, :], 3)
            c8 = constp.tile([128, 1], U32)
            nc.vector.memset(c8[:, :], 8)
            cFF = constp.tile([128, 1], U32)
            nc.vector.memset(cFF[:, :], 0xFF)

            rhs1 = bigp.tile([128, N], F32R)
            rhs2 = bigp.tile([66, N], F32R)
            lhsT1 = bigp.tile([128, n_queries], F32R)
            lhsT2 = bigp.tile([66, n_queries], F32R)
            vall = bigp.tile([128, m_tiles * 24], F32)
            pall = bigp.tile([128, m_tiles * 16], U32)
            outbuf = bigp.tile([128, m_tiles * K_OUT], U32)

            with (
                tc.tile_pool(name="stage", bufs=3) as stagep,
                tc.tile_pool(name="spool", bufs=2) as spool,
                tc.tile_pool(name="cpool", bufs=3) as cpool,
                tc.tile_pool(name="pmm", bufs=2, space="PSUM") as pmm,
            ):
                # Derive the operand tiles from minimal DMA traffic: one
                # [66, BLK] staged load per support chunk feeds the f32r
                # cast of bh (twice, for the K=128 duplication), the sq rows,
                # and the on-chip lo part bl = b - bh (gpsimd, idle at head).
                def load_support_chunk(cc):
                    sl = slice(cc * 1024, (cc + 1) * 1024)
                    st = stagep.tile([66, 1024], F32, tag="sb")
                    nc.sync.dma_start(st[:, :], bnsd.ap()[:, sl])
                    nc.scalar.copy(rhs1[0:64, sl], st[0:64, :])    # bh
                    nc.scalar.copy(rhs1[64:128, sl], st[0:64, :])  # bh dup
                    nc.scalar.copy(rhs2[64:66, sl], st[64:66, :])  # -sqh,-sql
                    nc.vector.scalar_tensor_tensor(
                        rhs2[0:64, sl],
                        rhs1[0:64, sl].bitcast(F32),
                        -1.0,
                        st[0:64, :],
                        mybir.AluOpType.mult,
                        mybir.AluOpType.add,
                    )  # bl = b - bh

                def load_query_chunk(cc):
                    qsl = slice(cc * 1024, (cc + 1) * 1024)
                    st = stagep.tile([128, 1024], F32, tag="sl1")
                    nc.scalar.dma_start(st[:, :], l1d.ap()[:, qsl])
                    nc.scalar.copy(lhsT1[:, qsl], st[:, :])        # 2ah; 2al
                    nc.scalar.copy(lhsT2[0:64, qsl], st[0:64, :])  # 2ah
                # batched decode: global = ((slot>>3)<<8) | (255 - (bits&0xFF))
                # 255 - (bits & 0xFF) == (bits ^ 0xFF) & 0xFF; base has low
                # 8 bits zero so add == bitwise or. Runs in quarters so the
                # earlier output DMAs overlap the main loop.
                base = bigp.tile([128, m_tiles * K_OUT], U32)
                lowb = bigp.tile([128, m_tiles * K_OUT], U32)

                def emit_decode(t0, t1):
                    ts = slice(t0, t1)
                    js = slice(t0 * K_OUT, t1 * K_OUT)
                    base_v = base[:, :].rearrange("p (t j) -> p t j", j=K_OUT)
                    lowb_v = lowb[:, :].rearrange("p (t j) -> p t j", j=K_OUT)
                    pall_v = pall[:, :].rearrange("p (t x) -> p t x", x=16)
                    vbits_v = (
                        vall[:, :]
                        .bitcast(U32)
                        .rearrange("p (t x) -> p t x", x=24)[:, ts, 0:17:2]
                    )
                    nc.vector.tensor_scalar(
                        base_v[:, ts, :],
                        pall_v[:, ts, 0:K_OUT],
                        c3[:, :],
                        c8[:, :],
                        mybir.AluOpType.logical_shift_right,
                        op1=mybir.AluOpType.logical_shift_left,
                    )
                    nc.vector.tensor_scalar(
                        lowb_v[:, ts, :],
                        vbits_v,
                        cFF[:, :],
                        cFF[:, :],
                        mybir.AluOpType.bitwise_xor,
                        op1=mybir.AluOpType.bitwise_and,
                    )
                    nc.vector.tensor_tensor(
                        outbuf[:, js], base[:, js], lowb[:, js],
                        mybir.AluOpType.bitwise_or,
                    )
                    nc.sync.dma_start(
                        out.ap().rearrange("(t p) j -> p t j", p=128)[:, ts, :],
                        outbuf[:, js].bitcast(I32).rearrange(
                            "p (t j) -> p t j", j=K_OUT
                        ),
                    )

                # first operand chunks before tile 0 can start; query chunk
                # and support chunks 0-1 first so tile 0's quarter 0 is
                # unblocked after just 5 casts.
                load_support_chunk(0)
                load_query_chunk(0)
                load_support_chunk(1)
                load_support_chunk(2)
                load_support_chunk(3)
                nc.sync.dma_start(
                    lhsT2[64:66, :]
                    .bitcast(F32)
                    .rearrange("p (r c) -> p r c", c=512),
                    ones2[:, :].unsqueeze(1).broadcast_to(
                        [2, n_queries // 512, 512]
                    ),
                )

                for t in range(m_tiles):
                    if t in (1, 3, 5):
                        load_query_chunk((t + 1) // 2)
                    qsl = slice(t * 128, (t + 1) * 128)
                    y = spool.tile([128, N], F32, tag="y")
                    cand = cpool.tile([128, 256], F32, tag="cand")
                    for q in range(N_BLK):
                        if t == 0 and q in (1, 2):
                            load_support_chunk(2 * q + 2)
                            load_support_chunk(2 * q + 3)
                        pq = pmm.tile([128, BLK], F32, tag="pq")
                        for c in range(BLK // 512):
                            sl = slice(
                                q * BLK + c * 512, q * BLK + (c + 1) * 512
                            )
                            psl = slice(c * 512, (c + 1) * 512)
                            nc.tensor.matmul(
                                pq[:, psl],
                                lhsT1[:, qsl],
                                rhs1[:, sl],
                                start=True,
                                stop=False,
                            )
                            nc.tensor.matmul(
                                pq[:, psl],
                                lhsT2[:, qsl],
                                rhs2[:, sl],
                                start=False,
                                stop=True,
                            )
                        ysl = y[:, q * BLK : (q + 1) * BLK]
                        nc.scalar.activation(
                            ysl,
                            pq[:, :],
                            mybir.ActivationFunctionType.Exp,
                            bias=bias_t[:, :],
                            scale=1.0,
                        )
                        if t in (0, m_tiles - 1):
                            # finer-grained stamp+scan shortens head and tail
                            b0 = (
                                ysl.bitcast(U8)
                                .rearrange("p (n four) -> p n four", four=4)
                                [:, :, 0]
                            )
                            nc.gpsimd.iota(
                                b0.rearrange("p (a b) -> p a b", b=CHUNK),
                                pattern=[[0, BLK // CHUNK], [-1, CHUNK]],
                                base=255,
                                channel_multiplier=0,
                                allow_small_or_imprecise_dtypes=True,
                            )
                            for ck in range(
                                q * (BLK // CHUNK), (q + 1) * (BLK // CHUNK)
                            ):
                                nc.vector.max(
                                    cand[:, ck * 8 : (ck + 1) * 8],
                                    y[:, ck * CHUNK : (ck + 1) * CHUNK],
                                )
                    if 0 < t < m_tiles - 1:
                        # stamp byte0 of each fp32 with (255-li), li in 0..255
                        # (one iota per tile: fewer Q7 launches and sem waits)
                        b0 = (
                            y[:, :]
                            .bitcast(U8)
                            .rearrange("p (n four) -> p n four", four=4)
                            [:, :, 0]
                        )
                        nc.gpsimd.iota(
                            b0.rearrange("p (a b) -> p a b", b=CHUNK),
                            pattern=[[0, N_CHUNKS], [-1, CHUNK]],
                            base=255,
                            channel_multiplier=0,
                            allow_small_or_imprecise_dtypes=True,
                        )
                        for ck in range(N_CHUNKS):
                            nc.vector.max(
                                cand[:, ck * 8 : (ck + 1) * 8],
                                y[:, ck * CHUNK : (ck + 1) * CHUNK],
                            )

                    # 3 extraction rounds; match_replace into fresh buffers so
                    # the original cand stays intact for the index lookups.
                    cand2 = cpool.tile([128, 256], F32, tag="cand2")
                    cand3 = cpool.tile([128, 256], F32, tag="cand3")
                    v0 = slice(t * 24, t * 24 + 8)
                    v1 = slice(t * 24 + 8, t * 24 + 16)
                    v2 = slice(t * 24 + 16, t * 24 + 24)
                    nc.vector.max(vall[:, v0], cand[:, :])
                    nc.vector.match_replace(
                        cand2[:, :], vall[:, v0], cand[:, :], NEG_BIG
                    )
                    nc.vector.max(vall[:, v1], cand2[:, :])
                    nc.vector.match_replace(
                        cand3[:, :], vall[:, v1], cand2[:, :], NEG_BIG
                    )
                    nc.vector.max(vall[:, v2], cand3[:, :])
                    # slots for the 9 needed ranks: {0,2,...,14} then 16..23
                    nc.vector.max_index(
                        pall[:, t * 16 : t * 16 + 8],
                        vall[:, t * 24 : t * 24 + 15 : 2],
                        cand[:, :],
                    )
                    nc.vector.max_index(
                        pall[:, t * 16 + 8 : t * 16 + 16],
                        vall[:, v2],
                        cand[:, :],
                    )
                    if t in (7, 15, 23):
                        emit_decode(t - 7, t + 1)
                    elif t == m_tiles - 2:
                        emit_decode(24, m_tiles - 1)
                    elif t == m_tiles - 1:
                        emit_decode(m_tiles - 1, m_tiles)

    return nc


_COMPILED = None


def _get_compiled():
    global _COMPILED
    if _COMPILED is None:
        _install_ntff_shim()
        import concourse.bacc as bacc

        nc = bacc.Bacc("TRN2", target_bir_lowering=False, debug=False)
        build_kernel(nc)
        nc.compile()
        _COMPILED = nc
    return _COMPILED


LAST_RESULTS = None

_HI_MASK = np.uint32(0xFFFFE000)  # keep 10 explicit mantissa bits


def _split_hi_lo(x: np.ndarray):
    """Exact hi/lo split: hi has low 13 mantissa bits zeroed, hi + lo == x."""
    x = np.ascontiguousarray(x, dtype=np.float32)
    hi = (x.view(np.uint32) & _HI_MASK).view(np.float32)
    return hi, x - hi


def kernel(query: np.ndarray, _trace=False, _tmpdir=None) -> np.ndarray:
    global LAST_RESULTS
    from concourse import bass_utils

    query = np.ascontiguousarray(query, dtype=np.float32)
    assert query.shape == (B, N, C), query.shape
    nc = _get_compiled()

    in_maps = []
    qT = np.ascontiguousarray(query.transpose(0, 2, 1))  # [B, C, N]
    for b in range(B):
        bt = qT[b]                                   # [C, N]
        sq = np.sum(bt * bt, axis=0, dtype=np.float32)
        sqh, sql = _split_hi_lo(sq)
        bns = np.ascontiguousarray(
            np.concatenate([bt, -sqh[None], -sql[None]], 0)
        )                                                            # [66, N]
        ah, al = _split_hi_lo(bt)
        l1f = np.concatenate([2.0 * ah, 2.0 * al], 0)                # [128,N]
        for h in range(2):
            csl = slice(h * NQ, (h + 1) * NQ)
            in_maps.append(
                {
                    "bns": bns,
                    "l1": np.ascontiguousarray(l1f[:, csl]),
                }
            )
    res = bass_utils.run_bass_kernel_spmd(
        nc, in_maps, core_ids=list(range(N_CORES)), trace=_trace, tmpdir=_tmpdir
    )
    LAST_RESULTS = res
    out = np.empty((B, N, K_OUT), np.int32)
    for core in range(N_CORES):
        b, h = divmod(core, 2)
        out[b, h * NQ : (h + 1) * NQ, :] = res.results[core]["idx"]
    return out


# revision 46
# speedup vs baseline: 1.1360x; 1.1360x over previous
"""Dilated KNN (k=9, dilation=2) over query[4, 8192, 64] on 8 NeuronCores.

Sharding: batch b and query-half h per core (core = 2*b + h). Each core
computes scores s[m, n] = 2*x_m.x_n - |x_n|^2 for its 4096 queries against
all 8192 supports of its batch (same ranking as negated squared euclidean
distance), selects the top-17 per row, and emits indices of ranks
0, 2, ..., 16.

Host prep: the query-side hi/lo mantissa split (11/13 bits, exact
2ah+2al==2a) and the support norms (fp32 sum + hi/lo split) are computed in
numpy; the device setup is chunked DMA + one-pass ACT f32r casts, with the
support lo part (bl = b - bh) derived on-chip, so the main loop starts
within ~20us. (The PE's f32r rounding is coarse (~tf32): collapsing the
hi/lo split to a single K=64 pass was measured to fail the accuracy gate.)

Single-DVE-pass top-k ("iota-stamp"):
  PE   : fp32r hi/lo split matmuls (exact products, fp32 PSUM accumulate)
         MM1: [2ah; 2al] . [bh; bh]          (K=128)
         MM2: [2ah; 1; 1] . [bl; -sqh; -sql] (K=66, drops 2*al.bl ~ 1e-6)
  ACT  : evicts PSUM through a monotone Exp map y = exp(s - 42.8), so the
         fp32 value order equals the score order with uniform absolute
         resolution ~2^-23 in score units.
  Pool : gpsimd iota overwrites byte 0 of every fp32 y with (255 - li),
         li = column index within a 256-wide chunk. Ranking resolution
         drops to ~3e-5 score units (fine: adjacent top-17 gaps are ~1e-1),
         and every candidate now carries its position in its low bits.
  DVE  : one max8 per 256-chunk (32/tile) -> 256 candidates with embedded
         positions; 3 merge rounds (max8 + match_replace into fresh buffers)
         give the top-24; two max_index calls over the original candidates
         recover the chunks for the 9 ranks the dilated output needs
         (0,2,...,16): call 1 a stride-2 view of ranks 0..14, call 2 ranks
         16..23.
  Decode (batched, quarter of the tiles at a time):
         global = ((slot >> 3) << 8) + 255 - (bits & 0xFF).
"""

import sys
import types

import numpy as np

B = 4
N = 8192
C = 64
K_OUT = 9
NQ = N // 2
N_CORES = 8
CHUNK = 256          # max8 scan chunk == iota stamp period
N_CHUNKS = N // CHUNK
NEG_BIG = -1.0e38
EXP_SHIFT = 42.8     # y = exp(s - 42.8); relevant scores s in [-25, 111]
BLK = 2048           # PSUM eviction block (columns per ACT op)
N_BLK = N // BLK
LOAD_CHUNK = 512


def _install_ntff_shim():
    """bass_utils imports antenv.axon_hooks for trace=True; the agent image
    lacks it. Register the ctypes-based hook so NTFF profiling works."""
    if "antenv.axon_hooks" in sys.modules:
        return
    try:
        from trn_agent_boot.trn_boot import _ntff_profile_via_ctypes

        hook = _ntff_profile_via_ctypes("/opt/axon/libaxon_pjrt.so")
        m = types.ModuleType("antenv.axon_hooks")
        m.get_axon_ntff_profile_hook = lambda: hook
        sys.modules["antenv.axon_hooks"] = m
    except Exception:
        pass


def build_kernel(nc, n_queries=NQ):
    import concourse.mybir as mybir
    import concourse.tile as tile

    F32 = mybir.dt.float32
    F32R = mybir.dt.float32r
    U32 = mybir.dt.uint32
    U8 = mybir.dt.uint8
    I32 = mybir.dt.int32

    m_tiles = n_queries // 128
    # bns rows 0..63: raw support features b; row 64: -sqh; row 65: -sql
    bnsd = nc.dram_tensor("bns", [66, N], F32, kind="ExternalInput")
    # l1 rows 0..63: 2*ah; rows 64..127: 2*al (host 11-bit hi/lo split)
    l1d = nc.dram_tensor("l1", [128, n_queries], F32, kind="ExternalInput")
    out = nc.dram_tensor("idx", [n_queries, K_OUT], I32, kind="ExternalOutput")

    with tile.TileContext(nc) as tc:
        with (
            tc.tile_pool(name="const", bufs=1) as constp,
            tc.tile_pool(name="big", bufs=1) as bigp,
        ):
            bias_t = constp.tile([128, 1], F32)
            nc.vector.memset(bias_t[:, :], -EXP_SHIFT)
            ones2 = constp.tile([2, 512], F32)
            nc.vector.memset(ones2[:, :], 1.0)
            c3 = constp.tile([128, 1], U32)
            nc.vector.memset(c3[:, :], 3)
            c8 = constp.tile([128, 1], U32)
            nc.vector.memset(c8[:, :], 8)
            cFF = constp.tile([128, 1], U32)
            nc.vector.memset(cFF[:, :], 0xFF)

            rhs1 = bigp.tile([128, N], F32R)
            rhs2 = bigp.tile([66, N], F32R)
            lhsT1 = bigp.tile([128, n_queries], F32R)
            lhsT2 = bigp.tile([66, n_queries], F32R)
            vall = bigp.tile([128, m_tiles * 24], F32)
            pall = bigp.tile([128, m_tiles * 16], U32)
            outbuf = bigp.tile([128, m_tiles * K_OUT], U32)

            with (
                tc.tile_pool(name="stage", bufs=3) as stagep,
                tc.tile_pool(name="spool", bufs=2) as spool,
                tc.tile_pool(name="cpool", bufs=3) as cpool,
                tc.tile_pool(name="pmm", bufs=2, space="PSUM") as pmm,
            ):
                # Derive the operand tiles from minimal DMA traffic: one
                # [66, BLK] staged load per support chunk feeds the f32r
                # cast of bh (twice, for the K=128 duplication), the sq rows,
                # and the on-chip lo part bl = b - bh (gpsimd, idle at head).
                def load_support_chunk(cc):
                    sl = slice(cc * 1024, (cc + 1) * 1024)
                    st = stagep.tile([66, 1024], F32, tag="sb")
                    nc.sync.dma_start(st[:, :], bnsd.ap()[:, sl])
                    nc.scalar.copy(rhs1[0:64, sl], st[0:64, :])    # bh
                    nc.scalar.copy(rhs1[64:128, sl], st[0:64, :])  # bh dup
                    nc.scalar.copy(rhs2[64:66, sl], st[64:66, :])  # -sqh,-sql
                    nc.vector.scalar_tensor_tensor(
                        rhs2[0:64, sl],
                        rhs1[0:64, sl].bitcast(F32),
                        -1.0,
                        st[0:64, :],
                        mybir.AluOpType.mult,
                        mybir.AluOpType.add,
                    )  # bl = b - bh

                def load_query_chunk(cc):
                    qsl = slice(cc * 1024, (cc + 1) * 1024)
                    st = stagep.tile([128, 1024], F32, tag="sl1")
                    nc.scalar.dma_start(st[:, :], l1d.ap()[:, qsl])
                    nc.scalar.copy(lhsT1[:, qsl], st[:, :])        # 2ah; 2al
                    nc.scalar.copy(lhsT2[0:64, qsl], st[0:64, :])  # 2ah
                # batched decode: global = ((slot>>3)<<8) | (255 - (bits&0xFF))
                # 255 - (bits & 0xFF) == (bits ^ 0xFF) & 0xFF; base has low
                # 8 bits zero so add == bitwise or. Runs in quarters so the
                # earlier output DMAs overlap the main loop.
                base = bigp.tile([128, m_tiles * K_OUT], U32)
                lowb = bigp.tile([128, m_tiles * K_OUT], U32)

                def emit_decode(t0, t1):
                    ts = slice(t0, t1)
                    js = slice(t0 * K_OUT, t1 * K_OUT)
                    base_v = base[:, :].rearrange("p (t j) -> p t j", j=K_OUT)
                    lowb_v = lowb[:, :].rearrange("p (t j) -> p t j", j=K_OUT)
                    pall_v = pall[:, :].rearrange("p (t x) -> p t x", x=16)
                    vbits_v = (
                        vall[:, :]
                        .bitcast(U32)
                        .rearrange("p (t x) -> p t x", x=24)[:, ts, 0:17:2]
                    )
                    nc.vector.tensor_scalar(
                        base_v[:, ts, :],
                        pall_v[:, ts, 0:K_OUT],
                        c3[:, :],
                        c8[:, :],
                        mybir.AluOpType.logical_shift_right,
                        op1=mybir.AluOpType.logical_shift_left,
                    )
                    nc.vector.tensor_scalar(
                        lowb_v[:, ts, :],
                        vbits_v,
                        cFF[:, :],
                        cFF[:, :],
                        mybir.AluOpType.bitwise_xor,
                        op1=mybir.AluOpType.bitwise_and,
                    )
                    nc.vector.tensor_tensor(
                        outbuf[:, js], base[:, js], lowb[:, js],
                        mybir.AluOpType.bitwise_or,
                    )
                    nc.sync.dma_start(
                        out.ap().rearrange("(t p) j -> p t j", p=128)[:, ts, :],
                        outbuf[:, js].bitcast(I32).rearrange(
                            "p (t j) -> p t j", j=K_OUT
                        ),
                    )

                # first operand chunks before tile 0 can start; query chunk
                # and support chunks 0-1 first so tile 0's quarter 0 is
                # unblocked after just 5 casts.
                load_support_chunk(0)
                load_query_chunk(0)
                load_support_chunk(1)
                load_support_chunk(2)
                load_support_chunk(3)
                nc.sync.dma_start(
                    lhsT2[64:66, :]
                    .bitcast(F32)
                    .rearrange("p (r c) -> p r c", c=512),
                    ones2[:, :].unsqueeze(1).broadcast_to(
                        [2, n_queries // 512, 512]
                    ),
                )

                for t in range(m_tiles):
                    if t in (1, 3, 5):
                        load_query_chunk((t + 1) // 2)
                    qsl = slice(t * 128, (t + 1) * 128)
                    y = spool.tile([128, N], F32, tag="y")
                    cand = cpool.tile([128, 256], F32, tag="cand")
                    for q in range(N_BLK):
                        if t == 0 and q in (1, 2):
                            load_support_chunk(2 * q + 2)
                            load_support_chunk(2 * q + 3)
                        pq = pmm.tile([128, BLK], F32, tag="pq")
                        for c in range(BLK // 512):
                            sl = slice(
                                q * BLK + c * 512, q * BLK + (c + 1) * 512
                            )
                            psl = slice(c * 512, (c + 1) * 512)
                            nc.tensor.matmul(
                                pq[:, psl],
                                lhsT1[:, qsl],
                                rhs1[:, sl],
                                start=True,
                                stop=False,
                            )
                            nc.tensor.matmul(
                                pq[:, psl],
                                lhsT2[:, qsl],
                                rhs2[:, sl],
                                start=False,
                                stop=True,
                            )
                        ysl = y[:, q * BLK : (q + 1) * BLK]
                        nc.scalar.activation(
                            ysl,
                            pq[:, :],
                            mybir.ActivationFunctionType.Exp,
                            bias=bias_t[:, :],
                            scale=1.0,
                        )
                        if t in (0, m_tiles - 1):
                            # finer-grained stamp+scan shortens head and tail
                            b0 = (
                                ysl.bitcast(U8)
                                .rearrange("p (n four) -> p n four", four=4)
                                [:, :, 0]
                            )
                            nc.gpsimd.iota(
                                b0.rearrange("p (a b) -> p a b", b=CHUNK),
                                pattern=[[0, BLK // CHUNK], [-1, CHUNK]],
                                base=255,
                                channel_multiplier=0,
                                allow_small_or_imprecise_dtypes=True,
                            )
                            for ck in range(
                                q * (BLK // CHUNK), (q + 1) * (BLK // CHUNK)
                            ):
                                nc.vector.max(
                                    cand[:, ck * 8 : (ck + 1) * 8],
                                    y[:, ck * CHUNK : (ck + 1) * CHUNK],
                                )
                    if 0 < t < m_tiles - 1:
                        # stamp byte0 of each fp32 with (255-li), li in 0..255
                        # (half-tile split so the stamp overlaps this tile's
                        # own evictions; one iota/tile was measured worse --
                        # the serial chain exceeds what 2 y-buffers hide)
                        for h in range(2):
                            b0 = (
                                y[:, h * (N // 2) : (h + 1) * (N // 2)]
                                .bitcast(U8)
                                .rearrange("p (n four) -> p n four", four=4)
                                [:, :, 0]
                            )
                            nc.gpsimd.iota(
                                b0.rearrange("p (a b) -> p a b", b=CHUNK),
                                pattern=[[0, N_CHUNKS // 2], [-1, CHUNK]],
                                base=255,
                                channel_multiplier=0,
                                allow_small_or_imprecise_dtypes=True,
                            )
                        for ck in range(N_CHUNKS):
                            nc.vector.max(
                                cand[:, ck * 8 : (ck + 1) * 8],
                                y[:, ck * CHUNK : (ck + 1) * CHUNK],
                            )

                    # 3 extraction rounds; match_replace into fresh buffers so
                    # the original cand stays intact for the index lookups.
                    cand2 = cpool.tile([128, 256], F32, tag="cand2")
                    cand3 = cpool.tile([128, 256], F32, tag="cand3")
                    v0 = slice(t * 24, t * 24 + 8)
                    v1 = slice(t * 24 + 8, t * 24 + 16)
                    v2 = slice(t * 24 + 16, t * 24 + 24)
                    nc.vector.max(vall[:, v0], cand[:, :])
                    nc.vector.match_replace(
                        cand2[:, :], vall[:, v0], cand[:, :], NEG_BIG
                    )
                    nc.vector.max(vall[:, v1], cand2[:, :])
                    nc.vector.match_replace(
                        cand3[:, :], vall[:, v1], cand2[:, :], NEG_BIG
                    )
                    nc.vector.max(vall[:, v2], cand3[:, :])
                    # slots for the 9 needed ranks: {0,2,...,14} then 16..23
                    nc.vector.max_index(
                        pall[:, t * 16 : t * 16 + 8],
                        vall[:, t * 24 : t * 24 + 15 : 2],
                        cand[:, :],
                    )
                    nc.vector.max_index(
                        pall[:, t * 16 + 8 : t * 16 + 16],
                        vall[:, v2],
                        cand[:, :],
                    )
                    if t in (7, 15, 23):
                        emit_decode(t - 7, t + 1)
                    elif t == m_tiles - 2:
                        emit_decode(24, m_tiles - 1)
                    elif t == m_tiles - 1:
                        emit_decode(m_tiles - 1, m_tiles)

    return nc


_COMPILED = None


def _get_compiled():
    global _COMPILED
    if _COMPILED is None:
        _install_ntff_shim()
        import concourse.bacc as bacc

        nc = bacc.Bacc("TRN2", target_bir_lowering=False, debug=False)
        build_kernel(nc)
        nc.compile()
        _COMPILED = nc
    return _COMPILED


LAST_RESULTS = None

_HI_MASK = np.uint32(0xFFFFE000)  # keep 10 explicit mantissa bits


def _split_hi_lo(x: np.ndarray):
    """Exact hi/lo split: hi has low 13 mantissa bits zeroed, hi + lo == x."""
    x = np.ascontiguousarray(x, dtype=np.float32)
    hi = (x.view(np.uint32) & _HI_MASK).view(np.float32)
    return hi, x - hi


def kernel(query: np.ndarray, _trace=False, _tmpdir=None) -> np.ndarray:
    global LAST_RESULTS
    from concourse import bass_utils

    query = np.ascontiguousarray(query, dtype=np.float32)
    assert query.shape == (B, N, C), query.shape
    nc = _get_compiled()

    in_maps = []
    qT = np.ascontiguousarray(query.transpose(0, 2, 1))  # [B, C, N]
    for b in range(B):
        bt = qT[b]                                   # [C, N]
        sq = np.sum(bt * bt, axis=0, dtype=np.float32)
        sqh, sql = _split_hi_lo(sq)
        bns = np.ascontiguousarray(
            np.concatenate([bt, -sqh[None], -sql[None]], 0)
        )                                                            # [66, N]
        ah, al = _split_hi_lo(bt)
        l1f = np.concatenate([2.0 * ah, 2.0 * al], 0)                # [128,N]
        for h in range(2):
            csl = slice(h * NQ, (h + 1) * NQ)
            in_maps.append(
                {
                    "bns": bns,
                    "l1": np.ascontiguousarray(l1f[:, csl]),
                }
            )
    res = bass_utils.run_bass_kernel_spmd(
        nc, in_maps, core_ids=list(range(N_CORES)), trace=_trace, tmpdir=_tmpdir
    )
    LAST_RESULTS = res
    out = np.empty((B, N, K_OUT), np.int32)
    for core in range(N_CORES):
        b, h = divmod(core, 2)
        out[b, h * NQ : (h + 1) * NQ, :] = res.results[core]["idx"]
    return out


# revision 47
# speedup vs baseline: 1.2266x; 1.0797x over previous
"""Dilated KNN (k=9, dilation=2) over query[4, 8192, 64] on 8 NeuronCores.

Sharding: batch b and query-half h per core (core = 2*b + h). Each core
computes scores s[m, n] = 2*x_m.x_n - |x_n|^2 for its 4096 queries against
all 8192 supports of its batch (same ranking as negated squared euclidean
distance), selects the top-17 per row, and emits indices of ranks
0, 2, ..., 16.

Host prep: the query-side hi/lo mantissa split (11/13 bits, exact
2ah+2al==2a) and the support norms (fp32 sum + hi/lo split) are computed in
numpy; the device setup is chunked DMA + one-pass ACT f32r casts, with the
support lo part (bl = b - bh) derived on-chip, so the main loop starts
within ~20us. (The PE's f32r rounding is coarse (~tf32): collapsing the
hi/lo split to a single K=64 pass was measured to fail the accuracy gate.)

Single-DVE-pass top-k ("iota-stamp"):
  PE   : fp32r hi/lo split matmuls (exact products, fp32 PSUM accumulate)
         MM1: [2ah; 2al] . [bh; bh]          (K=128)
         MM2: [2ah; 1; 1] . [bl; -sqh; -sql] (K=66, drops 2*al.bl ~ 1e-6)
  ACT  : evicts PSUM through a monotone Exp map y = exp(s - 42.8), so the
         fp32 value order equals the score order with uniform absolute
         resolution ~2^-23 in score units.
  Pool : gpsimd iota overwrites byte 0 of every fp32 y with (255 - li),
         li = column index within a 256-wide chunk. Ranking resolution
         drops to ~3e-5 score units (fine: adjacent top-17 gaps are ~1e-1),
         and every candidate now carries its position in its low bits.
  DVE  : one max8 per 256-chunk (32/tile) -> 256 candidates with embedded
         positions; 3 merge rounds (max8 + match_replace into fresh buffers)
         give the top-24; two max_index calls over the original candidates
         recover the chunks for the 9 ranks the dilated output needs
         (0,2,...,16): call 1 a stride-2 view of ranks 0..14, call 2 ranks
         16..23.
  Decode (batched, quarter of the tiles at a time):
         global = ((slot >> 3) << 8) + 255 - (bits & 0xFF).
"""

import sys
import types

import numpy as np

B = 4
N = 8192
C = 64
K_OUT = 9
NQ = N // 2
N_CORES = 8
CHUNK = 256          # max8 scan chunk == iota stamp period
N_CHUNKS = N // CHUNK
NEG_BIG = -1.0e38
EXP_SHIFT = 42.8     # y = exp(s - 42.8); relevant scores s in [-25, 111]
BLK = 2048           # PSUM eviction block (columns per ACT op)
N_BLK = N // BLK
LOAD_CHUNK = 512


def _install_ntff_shim():
    """bass_utils imports antenv.axon_hooks for trace=True; the agent image
    lacks it. Register the ctypes-based hook so NTFF profiling works."""
    if "antenv.axon_hooks" in sys.modules:
        return
    try:
        from trn_agent_boot.trn_boot import _ntff_profile_via_ctypes

        hook = _ntff_profile_via_ctypes("/opt/axon/libaxon_pjrt.so")
        m = types.ModuleType("antenv.axon_hooks")
        m.get_axon_ntff_profile_hook = lambda: hook
        sys.modules["antenv.axon_hooks"] = m
    except Exception:
        pass


def build_kernel(nc, n_queries=NQ):
    import concourse.mybir as mybir
    import concourse.tile as tile

    F32 = mybir.dt.float32
    F32R = mybir.dt.float32r
    U32 = mybir.dt.uint32
    U8 = mybir.dt.uint8
    I32 = mybir.dt.int32

    m_tiles = n_queries // 128
    # bns rows 0..63: raw support features b; row 64: -sqh; row 65: -sql
    bnsd = nc.dram_tensor("bns", [66, N], F32, kind="ExternalInput")
    # l1 rows 0..63: 2*ah; rows 64..127: 2*al (host 11-bit hi/lo split)
    l1d = nc.dram_tensor("l1", [128, n_queries], F32, kind="ExternalInput")
    out = nc.dram_tensor("idx", [n_queries, K_OUT], I32, kind="ExternalOutput")

    with tile.TileContext(nc) as tc:
        with (
            tc.tile_pool(name="const", bufs=1) as constp,
            tc.tile_pool(name="big", bufs=1) as bigp,
        ):
            bias_t = constp.tile([128, 1], F32)
            nc.vector.memset(bias_t[:, :], -EXP_SHIFT)
            ones2 = constp.tile([2, 512], F32)
            nc.vector.memset(ones2[:, :], 1.0)
            c3 = constp.tile([128, 1], U32)
            nc.vector.memset(c3[:, :], 3)
            c8 = constp.tile([128, 1], U32)
            nc.vector.memset(c8[:, :], 8)
            cFF = constp.tile([128, 1], U32)
            nc.vector.memset(cFF[:, :], 0xFF)

            rhs1 = bigp.tile([128, N], F32R)
            rhs2 = bigp.tile([66, N], F32R)
            lhsT1 = bigp.tile([128, n_queries], F32R)
            lhsT2 = bigp.tile([66, n_queries], F32R)
            vall = bigp.tile([128, m_tiles * 24], F32)
            pall = bigp.tile([128, m_tiles * 16], U32)
            outbuf = bigp.tile([128, m_tiles * K_OUT], U32)

            with (
                tc.tile_pool(name="stage", bufs=3) as stagep,
                tc.tile_pool(name="spool", bufs=2) as spool,
                tc.tile_pool(name="cpool", bufs=3) as cpool,
                tc.tile_pool(name="pmm", bufs=2, space="PSUM") as pmm,
            ):
                # Derive the operand tiles from minimal DMA traffic: one
                # [66, BLK] staged load per support chunk feeds the f32r
                # cast of bh (twice, for the K=128 duplication), the sq rows,
                # and the on-chip lo part bl = b - bh (gpsimd, idle at head).
                def load_support_chunk(cc):
                    sl = slice(cc * 1024, (cc + 1) * 1024)
                    st = stagep.tile([66, 1024], F32, tag="sb")
                    nc.sync.dma_start(st[:, :], bnsd.ap()[:, sl])
                    nc.scalar.copy(rhs1[0:64, sl], st[0:64, :])    # bh
                    # dup copies the already-f32r-rounded bits; sq rows are
                    # host-pre-rounded to 11 bits (exact under PE trunc).
                    # Both run on the head-idle DVE at 2 elem/cycle.
                    nc.vector.tensor_copy(rhs1[64:128, sl], rhs1[0:64, sl])
                    nc.vector.tensor_copy(rhs2[64:66, sl], st[64:66, :])
                    nc.vector.scalar_tensor_tensor(
                        rhs2[0:64, sl],
                        rhs1[0:64, sl].bitcast(F32),
                        -1.0,
                        st[0:64, :],
                        mybir.AluOpType.mult,
                        mybir.AluOpType.add,
                    )  # bl = b - bh

                def load_query_chunk(cc):
                    qsl = slice(cc * 1024, (cc + 1) * 1024)
                    st = stagep.tile([128, 1024], F32, tag="sl1")
                    nc.scalar.dma_start(st[:, :], l1d.ap()[:, qsl])
                    nc.scalar.copy(lhsT1[:, qsl], st[:, :])        # 2ah; 2al
                    nc.scalar.copy(lhsT2[0:64, qsl], st[0:64, :])  # 2ah
                # batched decode: global = ((slot>>3)<<8) | (255 - (bits&0xFF))
                # 255 - (bits & 0xFF) == (bits ^ 0xFF) & 0xFF; base has low
                # 8 bits zero so add == bitwise or. Runs in quarters so the
                # earlier output DMAs overlap the main loop.
                base = bigp.tile([128, m_tiles * K_OUT], U32)
                lowb = bigp.tile([128, m_tiles * K_OUT], U32)

                def emit_decode(t0, t1):
                    ts = slice(t0, t1)
                    js = slice(t0 * K_OUT, t1 * K_OUT)
                    base_v = base[:, :].rearrange("p (t j) -> p t j", j=K_OUT)
                    lowb_v = lowb[:, :].rearrange("p (t j) -> p t j", j=K_OUT)
                    pall_v = pall[:, :].rearrange("p (t x) -> p t x", x=16)
                    vbits_v = (
                        vall[:, :]
                        .bitcast(U32)
                        .rearrange("p (t x) -> p t x", x=24)[:, ts, 0:17:2]
                    )
                    nc.vector.tensor_scalar(
                        base_v[:, ts, :],
                        pall_v[:, ts, 0:K_OUT],
                        c3[:, :],
                        c8[:, :],
                        mybir.AluOpType.logical_shift_right,
                        op1=mybir.AluOpType.logical_shift_left,
                    )
                    nc.vector.tensor_scalar(
                        lowb_v[:, ts, :],
                        vbits_v,
                        cFF[:, :],
                        cFF[:, :],
                        mybir.AluOpType.bitwise_xor,
                        op1=mybir.AluOpType.bitwise_and,
                    )
                    nc.vector.tensor_tensor(
                        outbuf[:, js], base[:, js], lowb[:, js],
                        mybir.AluOpType.bitwise_or,
                    )
                    nc.sync.dma_start(
                        out.ap().rearrange("(t p) j -> p t j", p=128)[:, ts, :],
                        outbuf[:, js].bitcast(I32).rearrange(
                            "p (t j) -> p t j", j=K_OUT
                        ),
                    )

                # first operand chunks before tile 0 can start; query chunk
                # and support chunks 0-1 first so tile 0's quarter 0 is
                # unblocked after just 5 casts.
                load_support_chunk(0)
                load_query_chunk(0)
                load_support_chunk(1)
                load_support_chunk(2)
                load_support_chunk(3)
                nc.sync.dma_start(
                    lhsT2[64:66, :]
                    .bitcast(F32)
                    .rearrange("p (r c) -> p r c", c=512),
                    ones2[:, :].unsqueeze(1).broadcast_to(
                        [2, n_queries // 512, 512]
                    ),
                )

                for t in range(m_tiles):
                    if t in (1, 3, 5):
                        load_query_chunk((t + 1) // 2)
                    qsl = slice(t * 128, (t + 1) * 128)
                    y = spool.tile([128, N], F32, tag="y")
                    cand = cpool.tile([128, 256], F32, tag="cand")
                    for q in range(N_BLK):
                        if t == 0 and q in (1, 2):
                            load_support_chunk(2 * q + 2)
                            load_support_chunk(2 * q + 3)
                        pq = pmm.tile([128, BLK], F32, tag="pq")
                        for c in range(BLK // 512):
                            sl = slice(
                                q * BLK + c * 512, q * BLK + (c + 1) * 512
                            )
                            psl = slice(c * 512, (c + 1) * 512)
                            nc.tensor.matmul(
                                pq[:, psl],
                                lhsT1[:, qsl],
                                rhs1[:, sl],
                                start=True,
                                stop=False,
                            )
                            nc.tensor.matmul(
                                pq[:, psl],
                                lhsT2[:, qsl],
                                rhs2[:, sl],
                                start=False,
                                stop=True,
                            )
                        ysl = y[:, q * BLK : (q + 1) * BLK]
                        nc.scalar.activation(
                            ysl,
                            pq[:, :],
                            mybir.ActivationFunctionType.Exp,
                            bias=bias_t[:, :],
                            scale=1.0,
                        )
                        if t in (0, m_tiles - 1):
                            # finer-grained stamp+scan shortens head and tail
                            b0 = (
                                ysl.bitcast(U8)
                                .rearrange("p (n four) -> p n four", four=4)
                                [:, :, 0]
                            )
                            nc.gpsimd.iota(
                                b0.rearrange("p (a b) -> p a b", b=CHUNK),
                                pattern=[[0, BLK // CHUNK], [-1, CHUNK]],
                                base=255,
                                channel_multiplier=0,
                                allow_small_or_imprecise_dtypes=True,
                            )
                            for ck in range(
                                q * (BLK // CHUNK), (q + 1) * (BLK // CHUNK)
                            ):
                                nc.vector.max(
                                    cand[:, ck * 8 : (ck + 1) * 8],
                                    y[:, ck * CHUNK : (ck + 1) * CHUNK],
                                )
                    if 0 < t < m_tiles - 1:
                        # stamp byte0 of each fp32 with (255-li), li in 0..255
                        # (half-tile split so the stamp overlaps this tile's
                        # own evictions; one iota/tile was measured worse --
                        # the serial chain exceeds what 2 y-buffers hide)
                        for h in range(2):
                            b0 = (
                                y[:, h * (N // 2) : (h + 1) * (N // 2)]
                                .bitcast(U8)
                                .rearrange("p (n four) -> p n four", four=4)
                                [:, :, 0]
                            )
                            nc.gpsimd.iota(
                                b0.rearrange("p (a b) -> p a b", b=CHUNK),
                                pattern=[[0, N_CHUNKS // 2], [-1, CHUNK]],
                                base=255,
                                channel_multiplier=0,
                                allow_small_or_imprecise_dtypes=True,
                            )
                        for ck in range(N_CHUNKS):
                            nc.vector.max(
                                cand[:, ck * 8 : (ck + 1) * 8],
                                y[:, ck * CHUNK : (ck + 1) * CHUNK],
                            )

                    # 3 extraction rounds; match_replace into fresh buffers so
                    # the original cand stays intact for the index lookups.
                    cand2 = cpool.tile([128, 256], F32, tag="cand2")
                    cand3 = cpool.tile([128, 256], F32, tag="cand3")
                    v0 = slice(t * 24, t * 24 + 8)
                    v1 = slice(t * 24 + 8, t * 24 + 16)
                    v2 = slice(t * 24 + 16, t * 24 + 24)
                    nc.vector.max(vall[:, v0], cand[:, :])
                    nc.vector.match_replace(
                        cand2[:, :], vall[:, v0], cand[:, :], NEG_BIG
                    )
                    nc.vector.max(vall[:, v1], cand2[:, :])
                    nc.vector.match_replace(
                        cand3[:, :], vall[:, v1], cand2[:, :], NEG_BIG
                    )
                    nc.vector.max(vall[:, v2], cand3[:, :])
                    # slots for the 9 needed ranks: {0,2,...,14} then 16..23
                    nc.vector.max_index(
                        pall[:, t * 16 : t * 16 + 8],
                        vall[:, t * 24 : t * 24 + 15 : 2],
                        cand[:, :],
                    )
                    nc.vector.max_index(
                        pall[:, t * 16 + 8 : t * 16 + 16],
                        vall[:, v2],
                        cand[:, :],
                    )
                    if t in (7, 15, 23):
                        emit_decode(t - 7, t + 1)
                    elif t == m_tiles - 2:
                        emit_decode(24, m_tiles - 1)
                    elif t == m_tiles - 1:
                        emit_decode(m_tiles - 1, m_tiles)

    return nc


_COMPILED = None


def _get_compiled():
    global _COMPILED
    if _COMPILED is None:
        _install_ntff_shim()
        import concourse.bacc as bacc

        nc = bacc.Bacc("TRN2", target_bir_lowering=False, debug=False)
        build_kernel(nc)
        nc.compile()
        _COMPILED = nc
    return _COMPILED


LAST_RESULTS = None

_HI_MASK = np.uint32(0xFFFFE000)  # keep 10 explicit mantissa bits


def _split_hi_lo(x: np.ndarray):
    """Exact hi/lo split: hi has low 13 mantissa bits zeroed, hi + lo == x."""
    x = np.ascontiguousarray(x, dtype=np.float32)
    hi = (x.view(np.uint32) & _HI_MASK).view(np.float32)
    return hi, x - hi


def kernel(query: np.ndarray, _trace=False, _tmpdir=None) -> np.ndarray:
    global LAST_RESULTS
    from concourse import bass_utils

    query = np.ascontiguousarray(query, dtype=np.float32)
    assert query.shape == (B, N, C), query.shape
    nc = _get_compiled()

    in_maps = []
    qT = np.ascontiguousarray(query.transpose(0, 2, 1))  # [B, C, N]
    for b in range(B):
        bt = qT[b]                                   # [C, N]
        sq = np.sum(bt * bt, axis=0, dtype=np.float32)
        sqh, sql = _split_hi_lo(sq)
        bns = np.ascontiguousarray(
            np.concatenate([bt, -sqh[None], -sql[None]], 0)
        )                                                            # [66, N]
        ah, al = _split_hi_lo(bt)
        l1f = np.concatenate([2.0 * ah, 2.0 * al], 0)                # [128,N]
        for h in range(2):
            csl = slice(h * NQ, (h + 1) * NQ)
            in_maps.append(
                {
                    "bns": bns,
                    "l1": np.ascontiguousarray(l1f[:, csl]),
                }
            )
    res = bass_utils.run_bass_kernel_spmd(
        nc, in_maps, core_ids=list(range(N_CORES)), trace=_trace, tmpdir=_tmpdir
    )
    LAST_RESULTS = res
    out = np.empty((B, N, K_OUT), np.int32)
    for core in range(N_CORES):
        b, h = divmod(core, 2)
        out[b, h * NQ : (h + 1) * NQ, :] = res.results[core]["idx"]
    return out
